# revision 17
# baseline (speedup 1.0000x reference)
"""IF spiking-neuron scan (charge / fire / hard-reset) on 8 Trainium2 cores.

Reference recurrence over t (elementwise on every [B, N] element):
    v = v + x_t
    s = (v - 1.0 >= 0)          # spike, 0.0/1.0
    v = (1 - s) * v             # hard reset to 0

Sharding: pure data parallel over the B*N = 262144 element dimension;
each of the 8 cores owns 32768 element chains [T=64, 32768] with zero
communication. Per core the elements live in SBUF as a [128, 256] f32
state tile; the 64-step scan runs locally. All arithmetic is fp32 and
bit-exact vs the reference (adds, compares, and mult-by-0/1 only).

Kernel structure (all compute on the DVE):
  per timestep, two interleaved half-width streams (A/B) so each op's
  producer is two instructions back and the DVE write-ack latency is
  hidden:
      u[A] = v[A] + x_t[A]        (tensor_tensor add)
      u[B] = v[B] + x_t[B]
      v[A] = (u[A] < 1) * u[A]    (scalar_tensor_tensor fused cmp+mult)
      v[B] = (u[B] < 1) * u[B]
  per block of timesteps, ONE tensor_scalar computes every spike at
  once (2x DVE mode) and writes uint8 0/1 directly:
      s[:] = (u_block >= 1)       -> uint8 spike block
  Spikes travel to DRAM as uint8 in [partition, t, f] layout (2 KiB
  contiguous per partition per block -> line-rate DMA, 4x less output
  traffic); the host casts back to float32. Timestep blocks are small
  at the start/end of the scan to shrink the pipeline fill/drain.
"""

import numpy as np

import concourse.bass as bass
import concourse.tile as tile
from concourse import bacc, mybir
from concourse.bass_utils import run_bass_kernel_spmd

T = 64
B = 32
N = 8192
NCORES = 8
PERCORE = (B * N) // NCORES  # 32768 element chains per core
P = 128                      # SBUF partitions
F = PERCORE // P             # 256 elements per partition
H = F // 2                   # half-width for the two interleaved streams

V_TH = 1.0

# timestep block sizes: small at the edges to cut pipeline fill/drain
BLOCKS = [1, 1, 2, 4] + [8] * 6 + [4, 2, 1, 1]
assert sum(BLOCKS) == T

_NC_CACHE = {}


def build_nc(blocks=None, spike_on_act=True):
    blocks = list(BLOCKS if blocks is None else blocks)
    # Bacc (not raw Bass): its compile() splits multi-wait sync conditions
    # into nop/event-semaphore prefixes — walrus accepts at most one sync
    # wait per hardware instruction.
    nc = bacc.Bacc("TRN2", target_bir_lowering=False, debug=False)
    x = nc.dram_tensor("x", [T, PERCORE], mybir.dt.float32, kind="ExternalInput").ap()
    y = nc.dram_tensor("y", [P, T, F], mybir.dt.uint8, kind="ExternalOutput").ap()

    # x: [T, P*F] -> [P, T, F]; per partition each timestep is a contiguous
    # 1 KiB run in DRAM. y is already [P, T, F]: per partition a block of
    # timesteps is one contiguous run.
    xr = x.rearrange("t (p f) -> p t f", p=P)

    with tile.TileContext(nc) as tc:
        with (
            tc.tile_pool(name="xin", bufs=4) as xpool,
            tc.tile_pool(name="sout", bufs=4) as spool,
            tc.tile_pool(name="ub", bufs=3) as ubpool,
            tc.tile_pool(name="zb", bufs=2) as zpool,
            tc.tile_pool(name="v", bufs=1) as vpool,
        ):
            v = vpool.tile([P, F], mybir.dt.float32)
            nc.vector.memset(v[:], 0.0)
            t0 = 0
            for tb in blocks:
                xt = xpool.tile([P, tb * F], mybir.dt.float32, tag="xin")
                nc.sync.dma_start(xt[:], xr[:, t0:t0 + tb, :])
                ub = ubpool.tile([P, tb * F], mybir.dt.float32, tag="ub")
                for ti in range(tb):
                    for h in range(2):
                        lo = ti * F + h * H
                        nc.vector.tensor_add(
                            ub[:, lo:lo + H], v[:, h * H:(h + 1) * H],
                            xt[:, lo:lo + H],
                        )
                    for h in range(2):
                        lo = ti * F + h * H
                        nc.vector.scalar_tensor_tensor(
                            v[:, h * H:(h + 1) * H], ub[:, lo:lo + H], V_TH,
                            ub[:, lo:lo + H],
                            mybir.AluOpType.is_lt, mybir.AluOpType.mult,
                        )
                st = spool.tile([P, tb * F], mybir.dt.uint8, tag="sout")
                if spike_on_act:
                    # Spike path on the otherwise-idle scalar engine, exact
                    # even when u == V_TH:  z = sign(V_TH - u) in {-1,0,1},
                    # r = relu(z) in {0,1}; r == 1 - s, host flips it back.
                    zt = zpool.tile([P, tb * F], mybir.dt.float32, tag="zb")
                    nc.scalar.activation(
                        zt[:], ub[:], mybir.ActivationFunctionType.Sign,
                        bias=V_TH, scale=-1.0,
                    )
                    nc.scalar.activation(
                        st[:], zt[:], mybir.ActivationFunctionType.Relu,
                    )
                else:
                    nc.vector.tensor_scalar(
                        st[:], ub[:], V_TH, None, mybir.AluOpType.is_ge
                    )
                nc.sync.dma_start(y[:, t0:t0 + tb, :], st[:])
                t0 += tb
    nc.compile()
    return nc


def _get_nc():
    if "nc" not in _NC_CACHE:
        _NC_CACHE["nc"] = build_nc()
    return _NC_CACHE["nc"]


def run_sharded(x_seq, trace=False, nc=None, spike_on_act=True, **kwargs):
    if nc is None:
        nc = _get_nc()
    x2 = np.ascontiguousarray(np.asarray(x_seq, dtype=np.float32)).reshape(T, B * N)
    in_maps = [
        {"x": np.ascontiguousarray(x2[:, c * PERCORE:(c + 1) * PERCORE])}
        for c in range(NCORES)
    ]
    res = run_bass_kernel_spmd(nc, in_maps, list(range(NCORES)), trace=trace, **kwargs)
    out = np.empty((T, B * N), dtype=np.float32)
    for c in range(NCORES):
        yc = np.asarray(res.results[c]["y"])          # [P, T, F] uint8
        r = yc.transpose(1, 0, 2).reshape(T, PERCORE)
        # device stores r = 1 - s on the spike_on_act path
        out[:, c * PERCORE:(c + 1) * PERCORE] = (1 - r) if spike_on_act else r
    return out.reshape(T, B, N), res


def kernel(x_seq):
    out, _ = run_sharded(x_seq)
    return out


# revision 20
# speedup vs baseline: 1.0304x; 1.0304x over previous
"""IF spiking-neuron scan (charge / fire / hard-reset) on 8 Trainium2 cores.

Reference recurrence over t (elementwise on every [B, N] element):
    v = v + x_t
    s = (v - 1.0 >= 0)          # spike, 0.0/1.0
    v = (1 - s) * v             # hard reset to 0

Sharding: pure data parallel over the B*N = 262144 element dimension;
each of the 8 cores owns 32768 element chains [T=64, 32768] with zero
communication. Per core the elements live in SBUF as a [128, 256] f32
state tile; the 64-step scan runs locally. All arithmetic is fp32 and
bit-exact vs the reference (adds, compares, and mult-by-0/1 only).

Kernel structure (all compute on the DVE):
  per timestep, two interleaved half-width streams (A/B) so each op's
  producer is two instructions back and the DVE write-ack latency is
  hidden:
      u[A] = v[A] + x_t[A]        (tensor_tensor add)
      u[B] = v[B] + x_t[B]
      v[A] = (u[A] < 1) * u[A]    (scalar_tensor_tensor fused cmp+mult)
      v[B] = (u[B] < 1) * u[B]
  per block of timesteps, ONE tensor_scalar computes every spike at
  once (2x DVE mode) and writes uint8 0/1 directly:
      s[:] = (u_block >= 1)       -> uint8 spike block
  Spikes travel to DRAM as uint8 in [partition, t, f] layout (2 KiB
  contiguous per partition per block -> line-rate DMA, 4x less output
  traffic); the host casts back to float32. Timestep blocks are small
  at the start/end of the scan to shrink the pipeline fill/drain.
"""

import numpy as np

import concourse.bass as bass
import concourse.tile as tile
from concourse import bacc, mybir
from concourse.bass_utils import run_bass_kernel_spmd

T = 64
B = 32
N = 8192
NCORES = 8
PERCORE = (B * N) // NCORES  # 32768 element chains per core
P = 128                      # SBUF partitions
F = PERCORE // P             # 256 elements per partition
H = F // 2                   # half-width for the two interleaved streams

V_TH = 1.0

# timestep block sizes: small at the edges to cut pipeline fill/drain
BLOCKS = [2, 2, 4] + [8] * 6 + [4, 2, 2]
assert sum(BLOCKS) == T

_NC_CACHE = {}


def build_nc(blocks=None, spike_on_act=True):
    blocks = list(BLOCKS if blocks is None else blocks)
    # Bacc (not raw Bass): its compile() splits multi-wait sync conditions
    # into nop/event-semaphore prefixes — walrus accepts at most one sync
    # wait per hardware instruction.
    nc = bacc.Bacc("TRN2", target_bir_lowering=False, debug=False)
    x = nc.dram_tensor("x", [T, PERCORE], mybir.dt.float32, kind="ExternalInput").ap()
    y = nc.dram_tensor("y", [P, T, F], mybir.dt.uint8, kind="ExternalOutput").ap()

    # x: [T, P*F] -> [P, T, F]; per partition each timestep is a contiguous
    # 1 KiB run in DRAM. y is already [P, T, F]: per partition a block of
    # timesteps is one contiguous run.
    xr = x.rearrange("t (p f) -> p t f", p=P)

    with tile.TileContext(nc) as tc:
        with (
            tc.tile_pool(name="xin", bufs=4) as xpool,
            tc.tile_pool(name="sout", bufs=4) as spool,
            tc.tile_pool(name="ub", bufs=3) as ubpool,
            tc.tile_pool(name="zb", bufs=2) as zpool,
            tc.tile_pool(name="v", bufs=1) as vpool,
        ):
            v = vpool.tile([P, F], mybir.dt.float32)
            nc.vector.memset(v[:], 0.0)
            t0 = 0
            for tb in blocks:
                xt = xpool.tile([P, tb * F], mybir.dt.float32, tag="xin")
                nc.sync.dma_start(xt[:], xr[:, t0:t0 + tb, :])
                ub = ubpool.tile([P, tb * F], mybir.dt.float32, tag="ub")
                for ti in range(tb):
                    for h in range(2):
                        lo = ti * F + h * H
                        nc.vector.tensor_add(
                            ub[:, lo:lo + H], v[:, h * H:(h + 1) * H],
                            xt[:, lo:lo + H],
                        )
                    for h in range(2):
                        lo = ti * F + h * H
                        nc.vector.scalar_tensor_tensor(
                            v[:, h * H:(h + 1) * H], ub[:, lo:lo + H], V_TH,
                            ub[:, lo:lo + H],
                            mybir.AluOpType.is_lt, mybir.AluOpType.mult,
                        )
                st = spool.tile([P, tb * F], mybir.dt.uint8, tag="sout")
                if spike_on_act:
                    # Spike path on the otherwise-idle scalar engine, exact
                    # even when u == V_TH:  z = sign(V_TH - u) in {-1,0,1},
                    # r = relu(z) in {0,1}; r == 1 - s, host flips it back.
                    zt = zpool.tile([P, tb * F], mybir.dt.float32, tag="zb")
                    nc.scalar.activation(
                        zt[:], ub[:], mybir.ActivationFunctionType.Sign,
                        bias=V_TH, scale=-1.0,
                    )
                    nc.scalar.activation(
                        st[:], zt[:], mybir.ActivationFunctionType.Relu,
                    )
                else:
                    nc.vector.tensor_scalar(
                        st[:], ub[:], V_TH, None, mybir.AluOpType.is_ge
                    )
                nc.sync.dma_start(y[:, t0:t0 + tb, :], st[:])
                t0 += tb
    nc.compile()
    return nc


def _get_nc():
    if "nc" not in _NC_CACHE:
        _NC_CACHE["nc"] = build_nc()
    return _NC_CACHE["nc"]


def run_sharded(x_seq, trace=False, nc=None, spike_on_act=True, **kwargs):
    if nc is None:
        nc = _get_nc()
    x2 = np.ascontiguousarray(np.asarray(x_seq, dtype=np.float32)).reshape(T, B * N)
    in_maps = [
        {"x": np.ascontiguousarray(x2[:, c * PERCORE:(c + 1) * PERCORE])}
        for c in range(NCORES)
    ]
    # A cold device occasionally reports NRT_EXEC_UNIT_UNRECOVERABLE on the
    # first execute and recovers on the next attempt; retry a couple times.
    for attempt in range(3):
        try:
            res = run_bass_kernel_spmd(
                nc, in_maps, list(range(NCORES)), trace=trace, **kwargs
            )
            break
        except Exception:  # jax.errors.JaxRuntimeError and friends
            if attempt == 2:
                raise
            import time
            time.sleep(2.0)
    out = np.empty((T, B * N), dtype=np.float32)
    for c in range(NCORES):
        yc = np.asarray(res.results[c]["y"])          # [P, T, F] uint8
        r = yc.transpose(1, 0, 2).reshape(T, PERCORE)
        # device stores r = 1 - s on the spike_on_act path
        out[:, c * PERCORE:(c + 1) * PERCORE] = (1 - r) if spike_on_act else r
    return out.reshape(T, B, N), res


def kernel(x_seq):
    out, _ = run_sharded(x_seq)
    return out


# revision 26
# speedup vs baseline: 1.0510x; 1.0200x over previous
"""IF spiking-neuron scan (charge / fire / hard-reset) on 8 Trainium2 cores.

Reference recurrence over t (elementwise on every [B, N] element):
    v = v + x_t
    s = (v - 1.0 >= 0)          # spike, 0.0/1.0
    v = (1 - s) * v             # hard reset to 0

Sharding: pure data parallel over the B*N = 262144 element dimension;
each of the 8 cores owns 32768 element chains [T=64, 32768] with zero
communication. Per core the elements live in SBUF as a [128, 256] f32
state tile; the 64-step scan runs locally. All arithmetic is fp32 and
bit-exact vs the reference (adds, compares, and mult-by-0/1 only).

Kernel structure:
  per timestep, two interleaved half-width streams (A/B) on the vector
  engine, so each op's producer is two instructions back and the DVE
  write-ack latency is hidden:
      u[A] = v[A] + x_t[A]        (tensor_tensor add)
      u[B] = v[B] + x_t[B]
      v[A] = (u[A] < 1) * u[A]    (scalar_tensor_tensor fused cmp+mult)
      v[B] = (u[B] < 1) * u[B]
  per block of timesteps, the otherwise-idle scalar engine derives the
  spikes from the batched u values with an exact two-op step function
  (exact even at u == V_TH):
      z = sign(V_TH - u)          in {-1, 0, 1}
      r = relu(z)                 in {0, 1},  r == 1 - s
  r travels to DRAM as uint8 in [partition, t, f] layout (2 KiB
  contiguous per partition per block -> line-rate DMA, 4x less output
  traffic); the host computes s = 1 - r in float32. Timestep blocks are
  small at the start/end of the scan to shrink the pipeline fill/drain.
"""

import numpy as np

import concourse.tile as tile
from concourse import bacc, mybir
from concourse.bass_utils import run_bass_kernel_spmd

T = 64
B = 32
N = 8192
NCORES = 8
PERCORE = (B * N) // NCORES  # 32768 element chains per core
P = 128                      # SBUF partitions
F = PERCORE // P             # 256 elements per partition
H = F // 2                   # half-width for the two interleaved streams

V_TH = 1.0

# timestep block sizes: small at the edges to cut pipeline fill/drain
BLOCKS = [2, 2, 4] + [8] * 6 + [4, 2, 2]
assert sum(BLOCKS) == T

_NC_CACHE = {}


def build_nc(blocks=None, spike_on_act=True):
    blocks = list(BLOCKS if blocks is None else blocks)
    # Bacc (not raw Bass): its compile() splits multi-wait sync conditions
    # into nop/event-semaphore prefixes — walrus accepts at most one sync
    # wait per hardware instruction.
    nc = bacc.Bacc("TRN2", target_bir_lowering=False, debug=False)
    x = nc.dram_tensor("x", [T, PERCORE], mybir.dt.float32, kind="ExternalInput").ap()
    y = nc.dram_tensor("y", [P, T, F], mybir.dt.uint8, kind="ExternalOutput").ap()

    # x: [T, P*F] -> [P, T, F]; per partition each timestep is a contiguous
    # 1 KiB run in DRAM. y is already [P, T, F]: per partition a block of
    # timesteps is one contiguous run.
    xr = x.rearrange("t (p f) -> p t f", p=P)

    with tile.TileContext(nc) as tc:
        with (
            tc.tile_pool(name="xin", bufs=4) as xpool,
            tc.tile_pool(name="sout", bufs=4) as spool,
            tc.tile_pool(name="ub", bufs=3) as ubpool,
            tc.tile_pool(name="zb", bufs=2) as zpool,
            tc.tile_pool(name="v", bufs=1) as vpool,
        ):
            v = vpool.tile([P, F], mybir.dt.float32)
            nc.vector.memset(v[:], 0.0)
            t0 = 0
            for tb in blocks:
                xt = xpool.tile([P, tb * F], mybir.dt.float32, tag="xin")
                nc.sync.dma_start(xt[:], xr[:, t0:t0 + tb, :])
                ub = ubpool.tile([P, tb * F], mybir.dt.float32, tag="ub")
                for ti in range(tb):
                    for h in range(2):
                        lo = ti * F + h * H
                        nc.vector.tensor_add(
                            ub[:, lo:lo + H], v[:, h * H:(h + 1) * H],
                            xt[:, lo:lo + H],
                        )
                    if t0 + ti == T - 1:
                        continue  # v after the final timestep is never read
                    for h in range(2):
                        lo = ti * F + h * H
                        nc.vector.scalar_tensor_tensor(
                            v[:, h * H:(h + 1) * H], ub[:, lo:lo + H], V_TH,
                            ub[:, lo:lo + H],
                            mybir.AluOpType.is_lt, mybir.AluOpType.mult,
                        )
                st = spool.tile([P, tb * F], mybir.dt.uint8, tag="sout")
                # Final blocks: the DVE is idle once its scan ends, while the
                # scalar engine still owes Sign+Relu for the last u values —
                # a pure tail. Computing those spikes on the DVE as
                # r = (u < 1) removes the ACT tail before the last DMA.
                last_two = t0 + tb > T - 5
                if spike_on_act and not last_two:
                    # Spike path on the otherwise-idle scalar engine, exact
                    # even when u == V_TH:  z = sign(V_TH - u) in {-1,0,1},
                    # r = relu(z) in {0,1}; r == 1 - s, host flips it back.
                    zt = zpool.tile([P, tb * F], mybir.dt.float32, tag="zb")
                    nc.scalar.activation(
                        zt[:], ub[:], mybir.ActivationFunctionType.Sign,
                        bias=V_TH, scale=-1.0,
                    )
                    nc.scalar.activation(
                        st[:], zt[:], mybir.ActivationFunctionType.Relu,
                    )
                else:
                    # r = (u < V_TH) == 1 - s, same polarity as the ACT path
                    nc.vector.tensor_scalar(
                        st[:], ub[:], V_TH, None, mybir.AluOpType.is_lt
                    )
                # outputs ride the scalar engine's HW-DGE ring so input
                # triggers never queue behind them on the SP ring
                nc.scalar.dma_start(y[:, t0:t0 + tb, :], st[:])
                t0 += tb
    nc.compile()
    return nc


def _get_nc():
    if "nc" not in _NC_CACHE:
        _NC_CACHE["nc"] = build_nc()
    return _NC_CACHE["nc"]


def run_sharded(x_seq, trace=False, nc=None, spike_on_act=True, **kwargs):
    if nc is None:
        nc = _get_nc()
    x2 = np.ascontiguousarray(np.asarray(x_seq, dtype=np.float32)).reshape(T, B * N)
    in_maps = [
        {"x": np.ascontiguousarray(x2[:, c * PERCORE:(c + 1) * PERCORE])}
        for c in range(NCORES)
    ]
    # A cold device occasionally reports NRT_EXEC_UNIT_UNRECOVERABLE on the
    # first execute and recovers on the next attempt; retry a couple times.
    for attempt in range(3):
        try:
            res = run_bass_kernel_spmd(
                nc, in_maps, list(range(NCORES)), trace=trace, **kwargs
            )
            break
        except Exception:  # jax.errors.JaxRuntimeError and friends
            if attempt == 2:
                raise
            import time
            time.sleep(2.0)
    out = np.empty((T, B * N), dtype=np.float32)
    for c in range(NCORES):
        yc = np.asarray(res.results[c]["y"])          # [P, T, F] uint8
        r = yc.transpose(1, 0, 2).reshape(T, PERCORE)
        # device stores r = 1 - s on every path
        out[:, c * PERCORE:(c + 1) * PERCORE] = 1 - r
    return out.reshape(T, B, N), res


def kernel(x_seq):
    out, _ = run_sharded(x_seq)
    return out
